# revision 46
# baseline (speedup 1.0000x reference)
"""Trainium2 Bass kernel for nn_DHGNNLayer (gnn_message_passing).

Math (from the reference):
    h   = relu(B1 @ x @ W1)            # [n_nodes, 128], B1 = COO incidence
    out = mean_e sigmoid((hw0[r_{2e}] + hw0[r_{2e+1}]) / 2)   # scalar
    where hw0 = relu(h) @ W2[:, 0]     # only column 0 is ever needed

Key facts used:
  - inc_cols == arange(NNZ)//2  -> every edge has exactly 2 nonzeros, deg == 2.
  - The node -> (core, window, lane) placement is free: sorting nodes by
    degree (desc) into 128-lane windows makes every window degree-homogeneous,
    so the edge->node segment-sum becomes a PSUM-accumulated stream of
    host-transposed x tiles against a CONSTANT identity stationary operand:
        psum[c, lane] += xgT_t[c, lane]   (tile t = t-th nnz of each lane)
    No one-hot G matrices to build or ship at all.

Strategy (8 cores, 1D node-partition parallelism, no collectives):
  Launch A: nodes sorted by degree desc; window w (128 nodes) -> core w%8,
    slot w//8.  Slot rj = max degree (shared across cores; all cores run an
    identical program).  Equal-rj slot runs (width<=4) are fused into one
    matmul group of FD = width*128; fp8 DoubleRow matmuls consume subtile
    pairs, an odd tail uses a plain fp8 matmul.  W1 strips (FD 512) + DVE
    relu + w2col matmul follow per 4 slots; hw0 strips DMA straight from
    PSUM to DRAM.
  Launch B: host gathers hw0[inc_rows] (free), device does
    sigmoid(0.5*(a+b)) and reduces; host combines 8 partial sums.
"""

import numpy as np
import ml_dtypes

N_NODES = 50000
N_EDGES = 200000
C = 128
NNZ = 2 * N_EDGES
NCORES = 8
BLK = 128                      # nodes per window
NWIN = 392                     # windows (= 50176 node slots)
NSLOT = NWIN // NCORES         # 49 slots per core
NODES_PAD = NWIN * BLK         # 50176
GW = 4                         # max slots fused into one matmul group
CHUNK_TARGET = 4 * 128 * 1024  # DMA chunk target bytes (~0.5MB)

_PROGS = {}
TRACE = False
LAST = {}


def _bacc():
    import concourse.bacc as bacc

    return bacc.Bacc("TRN2", target_bir_lowering=False, debug=False,
                     num_devices=NCORES)


def _make_groups(rjs):
    """Slot runs of equal rj, width <= GW -> (slot0, width, rj)."""
    groups = []
    j = 0
    while j < NSLOT:
        w = 1
        while (j + w < NSLOT and w < GW and rjs[j + w] == rjs[j]):
            w += 1
        groups.append((j, w, int(rjs[j])))
        j += w
    return groups


def _pair_blocks(groups):
    """Flat xgT column layout: per group, per pair, a contiguous block.

    Returns (blocks, ncol): blocks = list of
      (colstart, ncols, slot0, w, nsub, start, stop)
    where nsub in (1, 2) subtiles and start/stop are the PSUM accumulate
    flags for the group's matmul sequence.
    """
    blocks = []
    col = 0
    for (s0, w, rj) in groups:
        npair = rj // 2
        odd = rj % 2
        nmm = npair + odd
        for p in range(npair):
            blocks.append((col, 2 * w * BLK, s0, w, 2, p == 0,
                           p == nmm - 1))
            col += 2 * w * BLK
        if odd:
            blocks.append((col, w * BLK, s0, w, 1, nmm == 1, True))
            col += w * BLK
    return blocks, col


def _chunks_of(blocks):
    """Greedy-batch pair blocks into DMA chunks (bytes <= CHUNK_TARGET).
    First chunk is kept small so compute starts early.  All chunks stay
    resident in SBUF (no buffer reuse), so DMA is never gated on compute."""
    chunks = []
    cur = []
    cur_bytes = 0
    ramp = [32 * 1024, 128 * 1024, 384 * 1024]   # small leading chunks
    limit = ramp[0]
    for b in blocks:
        nbytes = b[1] * 128
        if cur and cur_bytes + nbytes > limit:
            chunks.append(cur)
            cur = []
            cur_bytes = 0
            limit = ramp[len(chunks)] if len(chunks) < len(ramp) \
                else CHUNK_TARGET
        cur.append(b)
        cur_bytes += nbytes
    if cur:
        chunks.append(cur)
    return chunks


def _build_prog_a(rjs):
    """Layer-1 program: identity-stationary segment-sum + W1 + relu +
    W2[:,0] per node window."""
    import concourse.mybir as mybir
    from concourse import tile

    dtb = mybir.dt.bfloat16
    dtf = mybir.dt.float32
    dt8 = mybir.dt.float8e4
    AF = mybir.ActivationFunctionType
    PM = mybir.MatmulPerfMode
    NFREE = NSLOT * BLK        # 6272 nodes per core

    groups = _make_groups(rjs)
    blocks, ncol = _pair_blocks(groups)
    chunks = _chunks_of(blocks)

    nc = _bacc()
    xg_d = nc.dram_tensor("xg", [128, ncol], dt8, kind="ExternalInput")
    i2_d = nc.dram_tensor("i2", [128, 2, 128], dt8, kind="ExternalInput")
    w1_d = nc.dram_tensor("w1", [C, C], dtb, kind="ExternalInput")
    w2c_d = nc.dram_tensor("w2c", [C, 1], dtb, kind="ExternalInput")
    hw0_d = nc.dram_tensor("hw0", [1, NFREE], dtf, kind="ExternalOutput")

    nchunks = len(chunks)
    with tile.TileContext(nc) as tc:
        with (
            tc.tile_pool(name="const", bufs=1) as constp,
            tc.tile_pool(name="xgp", bufs=1) as xgp,
            tc.tile_pool(name="rlp", bufs=4) as rlp,
            tc.tile_pool(name="ps_hx", bufs=3, space="PSUM") as ps_hx,
            tc.tile_pool(name="ps_h", bufs=3, space="PSUM") as ps_h,
            tc.tile_pool(name="ps_o", bufs=2, space="PSUM") as ps_o,
        ):
            # I2[p, i, m] = (p == m) fp8, shipped from the host (32KB) so the
            # first matmul's dependency is a plain DMA-write like the chunks.
            i2 = constp.tile([128, 2, 128], dt8)
            nc.sync.dma_start(i2[:], i2_d[:])

            # xg streams through two HWDGE rings (sync + scalar) so ring
            # issuance never caps DMA; all chunks are resident in SBUF.
            bufs = {}
            chunk_cols = [sum(b[1] for b in ch) for ch in chunks]
            chunk_col0 = [ch[0][0] for ch in chunks]

            def issue_chunk(ci, eng):
                cols = chunk_cols[ci]
                t = xgp.tile([128, cols], dt8, tag=f"xg{ci}")
                eng.dma_start(t[:], xg_d[:, chunk_col0[ci]:
                                         chunk_col0[ci] + cols])
                bufs[ci] = t

            issue_chunk(0, nc.sync)
            w1_sb = constp.tile([C, C], dtb)
            nc.sync.dma_start(w1_sb[:], w1_d[:])
            w2c_sb = constp.tile([C, 1], dtb)
            nc.sync.dma_start(w2c_sb[:], w2c_d[:])
            for ci in range(1, nchunks):
                issue_chunk(ci, nc.sync)

            # preload the scalar activation table during the DMA ramp
            scratch = constp.tile([1, 1], dtf)
            nc.vector.memset(scratch[:], 0.0)
            nc.scalar.activation(scratch[:], scratch[:], AF.Copy)

            hxT_sb = constp.tile([128, NFREE], dtb)
            hw0_sb = constp.tile([1, NFREE], dtf)

            next_strip = 0

            def strips_upto(limit):
                nonlocal next_strip
                while next_strip * 4 + 4 <= limit or \
                        (limit == NSLOT and next_strip * 4 < NSLOT):
                    s0 = next_strip * 4
                    fw = min(4, NSLOT - s0) * BLK
                    psh = ps_h.tile([C, 512], dtf, tag="h")
                    nc.tensor.matmul(psh[:, :fw], w1_sb[:],
                                     hxT_sb[:, s0 * BLK:s0 * BLK + fw],
                                     start=True, stop=True)
                    reluT = rlp.tile([128, 512], dtb, tag="reluT")
                    if next_strip % 2 == 0:
                        nc.scalar.activation(reluT[:, :fw], psh[:, :fw],
                                             AF.Relu)
                    else:
                        nc.vector.tensor_scalar(reluT[:, :fw], psh[:, :fw],
                                                0.0, None,
                                                mybir.AluOpType.max)
                    pso = ps_o.tile([1, 512], dtf, tag="o")
                    nc.tensor.matmul(pso[:, :fw], w2c_sb[:], reluT[:, :fw],
                                     start=True, stop=True)
                    nc.scalar.activation(hw0_sb[:, s0 * BLK:s0 * BLK + fw],
                                         pso[:, :fw], AF.Copy)
                    next_strip += 1

            # segment-sum matmuls, group copies alternating scalar/DVE
            ncopy = 0
            psum = None
            for ci, ch in enumerate(chunks):
                buf = bufs[ci]
                base = chunk_col0[ci]
                for (colstart, ncols, s0, w, nsub, start, stop) in ch:
                    off = colstart - base
                    fw = w * BLK
                    if start:
                        psum = ps_hx.tile([C, fw], dtf, tag="hx")
                    if nsub == 2:
                        rhs = buf[:, off:off + 2 * fw].rearrange(
                            "p (two f) -> p two f", two=2)
                        nc.tensor.matmul(psum[:], i2[:], rhs,
                                         start=start, stop=stop,
                                         perf_mode=PM.DoubleRow)
                    else:
                        nc.tensor.matmul(psum[:], i2[:, 0, :],
                                         buf[:, off:off + fw],
                                         start=start, stop=stop)
                    if stop:
                        nc.vector.tensor_copy(
                            out=hxT_sb[:, s0 * BLK:s0 * BLK + fw],
                            in_=psum[:])
                        ncopy += 1
                        strips_upto(s0 + w)
            strips_upto(NSLOT)
            nc.sync.dma_start(hw0_d[:], hw0_sb[:])

    nc.compile()
    return nc


def _build_prog_b(free):
    """Layer-2 program (raw bass, minimal tail):
    acc[p] = sum_f sigmoid(0.5*(a+b)).  zab is [za | zb] along free."""
    import concourse.bass as bass
    import concourse.mybir as mybir

    dtb = mybir.dt.bfloat16
    dtf = mybir.dt.float32
    AF = mybir.ActivationFunctionType

    nc = bass.Bass()
    zab_d = nc.dram_tensor("zab", [128, 2 * free], dtb, kind="ExternalInput")
    acc_d = nc.dram_tensor("acc", [128, 1], dtf, kind="ExternalOutput")

    with (
        nc.sbuf_tensor([128, 2 * free], dtb) as zab_sb,
        nc.sbuf_tensor([128, free], dtf) as t_sb,
        nc.sbuf_tensor([128, free], dtf) as s_sb,
        nc.sbuf_tensor([128, 1], dtf) as r_sb,
        nc.sbuf_tensor([1, 1], dtf) as scratch_sb,
        nc.semaphore() as dsem,
        nc.semaphore() as csem,
        nc.Block() as block,
    ):
        @block.sync
        def _(sync):
            sync.dma_start(zab_sb[:, :free],
                           zab_d[:, :free]).then_inc(dsem, 16)
            sync.wait_ge(csem, 6)
            sync.dma_start(acc_d[:], r_sb[:]).then_inc(dsem, 16)

        @block.vector
        def _(vector):
            nc.vector.memset(scratch_sb[:], 0.0).then_inc(csem, 4)
            vector.wait_ge(dsem, 32)
            nc.vector.tensor_add(t_sb[:], zab_sb[:, :free],
                                 zab_sb[:, free:]).then_inc(csem, 1)

        @block.scalar
        def _(scalar):
            # zb half on the scalar HWDGE ring: parallel issuance + queues
            nc.scalar.dma_start(zab_sb[:, free:],
                                zab_d[:, free:]).then_inc(dsem, 16)
            # preload the sigmoid act table while the DMA is in flight
            scalar.wait_ge(csem, 4)
            nc.scalar.activation(scratch_sb[:], scratch_sb[:], AF.Sigmoid)
            scalar.wait_ge(csem, 5)
            nc.scalar.activation(s_sb[:], t_sb[:], AF.Sigmoid, scale=0.5,
                                 accum_out=r_sb[:]).then_inc(csem, 1)

    return nc


def _get_prog(key, builder, *args):
    if key not in _PROGS:
        _PROGS[key] = builder(*args)
    return _PROGS[key]


def _run(nc, in_maps, tag):
    from concourse.bass_utils import run_bass_kernel_spmd
    import time

    t0 = time.perf_counter()
    res = run_bass_kernel_spmd(nc, in_maps, list(range(NCORES)), trace=TRACE)
    LAST[tag + "_wall_s"] = time.perf_counter() - t0
    LAST[tag + "_exec_ns"] = res.exec_time_ns
    return res.results


def kernel(x, w1, w2, inc_rows, inc_cols, n_nodes=None, n_edges=None):
    x = np.asarray(x, dtype=np.float32)
    w1 = np.asarray(w1, dtype=np.float32)
    w2 = np.asarray(w2, dtype=np.float32)
    inc_rows = np.asarray(inc_rows)
    inc_cols = np.asarray(inc_cols)
    assert x.shape == (N_EDGES, C) and inc_rows.shape == (NNZ,)
    assert np.array_equal(inc_cols.astype(np.int64),
                          np.arange(NNZ, dtype=np.int64) // 2)

    # ---- host prep: degree-sorted node placement ----
    rs = inc_rows.astype(np.int64)
    deg = np.bincount(rs, minlength=NODES_PAD)      # padded node space
    order = np.argsort(-deg, kind="stable")         # node rank by deg desc
    rank = np.empty(NODES_PAD, np.int64)
    rank[order] = np.arange(NODES_PAD)
    ds = deg[order]                                 # sorted degrees

    win = rank >> 7                                 # window of each node
    lane = rank & 127
    core_of = win % NCORES
    slot_of = win // NCORES

    rjs = ds.reshape(NWIN, BLK).max(1).reshape(NSLOT, NCORES).max(1)
    rjs = np.maximum(rjs, 1).astype(np.int64)

    groups = _make_groups(rjs)
    blocks, ncol = _pair_blocks(groups)

    # flat column start for (slot, t): where the t-th nnz block of each
    # slot's 128 lanes lives
    maxrj = int(rjs.max())
    slot_t_col = np.full((NSLOT, maxrj), -1, np.int64)
    for (s0, w, rj) in groups:
        # recompute this group's colbase from blocks: first block of group
        pass
    col = 0
    for (s0, w, rj) in groups:
        npair = rj // 2
        odd = rj % 2
        for p in range(npair):
            for i in range(2):
                t = 2 * p + i
                for ws in range(w):
                    slot_t_col[s0 + ws, t] = col + i * w * BLK + ws * BLK
            col += 2 * w * BLK
        if odd:
            t = rj - 1
            for ws in range(w):
                slot_t_col[s0 + ws, t] = col + ws * BLK
            col += w * BLK
    assert col == ncol

    # per-nnz placement: sort nnz by node to get within-node index t
    nnz_order = np.argsort(rs, kind="stable")
    rs_s = rs[nnz_order]
    cs_s = inc_cols.astype(np.int64)[nnz_order]
    starts = np.zeros(NODES_PAD, np.int64)
    starts[1:] = np.cumsum(deg)[:-1]
    t_k = np.arange(NNZ, dtype=np.int64) - starts[rs_s]

    core_k = core_of[rs_s]
    col_k = slot_t_col[slot_of[rs_s], t_k] + lane[rs_s]

    x8 = x.astype(ml_dtypes.float8_e4m3)
    XF = np.zeros((NCORES, ncol, C), dtype=ml_dtypes.float8_e4m3)
    XF[core_k, col_k, :] = x8[cs_s]
    XFT = np.ascontiguousarray(XF.transpose(0, 2, 1))   # [8, 128, ncol]

    w1b = w1.astype(ml_dtypes.bfloat16)
    w2cb = w2[:, 0:1].astype(ml_dtypes.bfloat16)
    eye = (np.arange(128)[:, None] == np.arange(128)[None, :])
    i2h = np.broadcast_to(eye[:, None, :], (128, 2, 128)).astype(
        ml_dtypes.float8_e4m3)
    i2h = np.ascontiguousarray(i2h)

    prog_a = _get_prog(("A", tuple(rjs.tolist())), _build_prog_a, rjs)
    in_maps = [{"xg": XFT[m], "i2": i2h, "w1": w1b, "w2c": w2cb}
               for m in range(NCORES)]
    res_a = _run(prog_a, in_maps, "A")

    # ---- host glue: assemble hw0, gather per-nonzero values ----
    parts = np.stack([res_a[m]["hw0"].reshape(-1) for m in range(NCORES)])
    # node n -> parts[core_of[n], slot_of[n]*128 + lane[n]]
    hw0 = parts[core_of, slot_of * BLK + lane]          # [NODES_PAD]
    zg = hw0[rs]
    za = zg[0::2]
    zb = zg[1::2]

    # ---- launch B: sigmoid + reduce ----
    FREE = -(-N_EDGES // (NCORES * 128))               # 196
    tot = NCORES * 128 * FREE
    zap = np.full(tot, -1.0e4, np.float32)
    zbp = np.full(tot, -1.0e4, np.float32)
    zap[:N_EDGES] = za
    zbp[:N_EDGES] = zb
    zab = np.concatenate(
        [zap.reshape(NCORES, 128, FREE), zbp.reshape(NCORES, 128, FREE)],
        axis=2).astype(ml_dtypes.bfloat16)

    prog_b = _get_prog(("B", FREE), _build_prog_b, FREE)
    in_maps_b = [{"zab": zab[m]} for m in range(NCORES)]
    res_b = _run(prog_b, in_maps_b, "B")

    total = float(sum(float(r["acc"].sum()) for r in res_b))
    return np.array(total / N_EDGES, dtype=np.float32)


# revision 47
# speedup vs baseline: 1.0930x; 1.0930x over previous
"""Trainium2 Bass kernel for nn_DHGNNLayer (gnn_message_passing).

Math (from the reference):
    h   = relu(B1 @ x @ W1)            # [n_nodes, 128], B1 = COO incidence
    out = mean_e sigmoid((hw0[r_{2e}] + hw0[r_{2e+1}]) / 2)   # scalar
    where hw0 = relu(h) @ W2[:, 0]     # only column 0 is ever needed

Key facts used:
  - inc_cols == arange(NNZ)//2  -> every edge has exactly 2 nonzeros, deg == 2.
  - The node -> (core, window, lane) placement is free: sorting nodes by
    degree (desc) into 128-lane windows makes every window degree-homogeneous,
    so the edge->node segment-sum becomes a PSUM-accumulated stream of
    host-transposed x tiles against a CONSTANT identity stationary operand:
        psum[c, lane] += xgT_t[c, lane]   (tile t = t-th nnz of each lane)
    No one-hot G matrices to build or ship at all.

Strategy (8 cores, 1D node-partition parallelism, no collectives):
  Launch A: nodes sorted by degree desc; window w (128 nodes) -> core w%8,
    slot w//8.  Slot rj = max degree (shared across cores; all cores run an
    identical program).  Equal-rj slot runs (width<=4) are fused into one
    matmul group of FD = width*128; fp8 DoubleRow matmuls consume subtile
    pairs, an odd tail uses a plain fp8 matmul.  W1 strips (FD 512) + DVE
    relu + w2col matmul follow per 4 slots; hw0 strips DMA straight from
    PSUM to DRAM.
  Launch B: host gathers hw0[inc_rows] (free), device does
    sigmoid(0.5*(a+b)) and reduces; host combines 8 partial sums.
"""

import numpy as np
import ml_dtypes

N_NODES = 50000
N_EDGES = 200000
C = 128
NNZ = 2 * N_EDGES
NCORES = 8
BLK = 128                      # nodes per window
NWIN = 392                     # windows (= 50176 node slots)
NSLOT = NWIN // NCORES         # 49 slots per core
NODES_PAD = NWIN * BLK         # 50176
GW = 4                         # max slots fused into one matmul group
CHUNK_TARGET = 4 * 128 * 1024  # DMA chunk target bytes (~0.5MB)

_PROGS = {}
TRACE = False
LAST = {}


def _bacc():
    import concourse.bacc as bacc

    return bacc.Bacc("TRN2", target_bir_lowering=False, debug=False,
                     num_devices=NCORES)


def _make_groups(rjs):
    """Slot runs of equal rj, width <= GW -> (slot0, width, rj)."""
    groups = []
    j = 0
    while j < NSLOT:
        w = 1
        while (j + w < NSLOT and w < GW and rjs[j + w] == rjs[j]):
            w += 1
        groups.append((j, w, int(rjs[j])))
        j += w
    return groups


def _pair_blocks(groups):
    """Flat xgT column layout: per group, per pair, a contiguous block.

    Returns (blocks, ncol): blocks = list of
      (colstart, ncols, slot0, w, nsub, start, stop)
    where nsub in (1, 2) subtiles and start/stop are the PSUM accumulate
    flags for the group's matmul sequence.
    """
    blocks = []
    col = 0
    for (s0, w, rj) in groups:
        npair = rj // 2
        odd = rj % 2
        nmm = npair + odd
        for p in range(npair):
            blocks.append((col, 2 * w * BLK, s0, w, 2, p == 0,
                           p == nmm - 1))
            col += 2 * w * BLK
        if odd:
            blocks.append((col, w * BLK, s0, w, 1, nmm == 1, True))
            col += w * BLK
    return blocks, col


def _chunks_of(blocks):
    """Greedy-batch pair blocks into DMA chunks (bytes <= CHUNK_TARGET).
    First chunk is kept small so compute starts early.  All chunks stay
    resident in SBUF (no buffer reuse), so DMA is never gated on compute."""
    chunks = []
    cur = []
    cur_bytes = 0
    ramp = [64 * 1024, 256 * 1024]   # small leading chunks: compute starts early
    limit = ramp[0]
    for b in blocks:
        nbytes = b[1] * 128
        if cur and cur_bytes + nbytes > limit:
            chunks.append(cur)
            cur = []
            cur_bytes = 0
            limit = ramp[len(chunks)] if len(chunks) < len(ramp) \
                else CHUNK_TARGET
        cur.append(b)
        cur_bytes += nbytes
    if cur:
        chunks.append(cur)
    return chunks


def _build_prog_a(rjs):
    """Layer-1 program: identity-stationary segment-sum + W1 + relu +
    W2[:,0] per node window."""
    import concourse.mybir as mybir
    from concourse import tile

    dtb = mybir.dt.bfloat16
    dtf = mybir.dt.float32
    dt8 = mybir.dt.float8e4
    AF = mybir.ActivationFunctionType
    PM = mybir.MatmulPerfMode
    NFREE = NSLOT * BLK        # 6272 nodes per core

    groups = _make_groups(rjs)
    blocks, ncol = _pair_blocks(groups)
    chunks = _chunks_of(blocks)

    nc = _bacc()
    xg_d = nc.dram_tensor("xg", [128, ncol], dt8, kind="ExternalInput")
    i2_d = nc.dram_tensor("i2", [128, 2, 128], dt8, kind="ExternalInput")
    w1_d = nc.dram_tensor("w1", [C, C], dtb, kind="ExternalInput")
    w2c_d = nc.dram_tensor("w2c", [C, 1], dtb, kind="ExternalInput")
    hw0_d = nc.dram_tensor("hw0", [1, NFREE], dtf, kind="ExternalOutput")

    nchunks = len(chunks)
    with tile.TileContext(nc) as tc:
        with (
            tc.tile_pool(name="const", bufs=1) as constp,
            tc.tile_pool(name="xgp", bufs=1) as xgp,
            tc.tile_pool(name="rlp", bufs=4) as rlp,
            tc.tile_pool(name="ps_hx", bufs=3, space="PSUM") as ps_hx,
            tc.tile_pool(name="ps_h", bufs=3, space="PSUM") as ps_h,
            tc.tile_pool(name="ps_o", bufs=2, space="PSUM") as ps_o,
        ):
            # I2[p, i, m] = (p == m) fp8, shipped from the host (32KB) so the
            # first matmul's dependency is a plain DMA-write like the chunks.
            i2 = constp.tile([128, 2, 128], dt8)
            nc.sync.dma_start(i2[:], i2_d[:])

            # xg streams through two HWDGE rings (sync + scalar) so ring
            # issuance never caps DMA; all chunks are resident in SBUF.
            bufs = {}
            chunk_cols = [sum(b[1] for b in ch) for ch in chunks]
            chunk_col0 = [ch[0][0] for ch in chunks]

            def issue_chunk(ci, eng):
                cols = chunk_cols[ci]
                t = xgp.tile([128, cols], dt8, tag=f"xg{ci}")
                eng.dma_start(t[:], xg_d[:, chunk_col0[ci]:
                                         chunk_col0[ci] + cols])
                bufs[ci] = t

            issue_chunk(0, nc.sync)
            w1_sb = constp.tile([C, C], dtb)
            nc.sync.dma_start(w1_sb[:], w1_d[:])
            w2c_sb = constp.tile([C, 1], dtb)
            nc.sync.dma_start(w2c_sb[:], w2c_d[:])
            for ci in range(1, nchunks):
                issue_chunk(ci, nc.sync)

            # preload the scalar activation table during the DMA ramp
            scratch = constp.tile([1, 1], dtf)
            nc.vector.memset(scratch[:], 0.0)
            nc.scalar.activation(scratch[:], scratch[:], AF.Copy)

            hxT_sb = constp.tile([128, NFREE], dtb)
            hw0_sb = constp.tile([1, NFREE], dtf)

            next_strip = 0

            def strips_upto(limit):
                nonlocal next_strip
                while next_strip * 4 + 4 <= limit or \
                        (limit == NSLOT and next_strip * 4 < NSLOT):
                    s0 = next_strip * 4
                    fw = min(4, NSLOT - s0) * BLK
                    psh = ps_h.tile([C, 512], dtf, tag="h")
                    nc.tensor.matmul(psh[:, :fw], w1_sb[:],
                                     hxT_sb[:, s0 * BLK:s0 * BLK + fw],
                                     start=True, stop=True)
                    reluT = rlp.tile([128, 512], dtb, tag="reluT")
                    if next_strip % 2 == 0:
                        nc.scalar.activation(reluT[:, :fw], psh[:, :fw],
                                             AF.Relu)
                    else:
                        nc.vector.tensor_scalar(reluT[:, :fw], psh[:, :fw],
                                                0.0, None,
                                                mybir.AluOpType.max)
                    pso = ps_o.tile([1, 512], dtf, tag="o")
                    nc.tensor.matmul(pso[:, :fw], w2c_sb[:], reluT[:, :fw],
                                     start=True, stop=True)
                    nc.scalar.activation(hw0_sb[:, s0 * BLK:s0 * BLK + fw],
                                         pso[:, :fw], AF.Copy)
                    next_strip += 1

            # segment-sum matmuls, group copies alternating scalar/DVE
            ncopy = 0
            psum = None
            for ci, ch in enumerate(chunks):
                buf = bufs[ci]
                base = chunk_col0[ci]
                for (colstart, ncols, s0, w, nsub, start, stop) in ch:
                    off = colstart - base
                    fw = w * BLK
                    if start:
                        psum = ps_hx.tile([C, fw], dtf, tag="hx")
                    if nsub == 2:
                        rhs = buf[:, off:off + 2 * fw].rearrange(
                            "p (two f) -> p two f", two=2)
                        nc.tensor.matmul(psum[:], i2[:], rhs,
                                         start=start, stop=stop,
                                         perf_mode=PM.DoubleRow)
                    else:
                        nc.tensor.matmul(psum[:], i2[:, 0, :],
                                         buf[:, off:off + fw],
                                         start=start, stop=stop)
                    if stop:
                        nc.vector.tensor_copy(
                            out=hxT_sb[:, s0 * BLK:s0 * BLK + fw],
                            in_=psum[:])
                        ncopy += 1
                        strips_upto(s0 + w)
            strips_upto(NSLOT)
            nc.sync.dma_start(hw0_d[:], hw0_sb[:])

    nc.compile()
    return nc


def _build_prog_b(free):
    """Layer-2 program (raw bass, minimal tail):
    acc[p] = sum_f sigmoid(0.5*(a+b)).  zab is [za | zb] along free."""
    import concourse.bass as bass
    import concourse.mybir as mybir

    dtb = mybir.dt.bfloat16
    dtf = mybir.dt.float32
    AF = mybir.ActivationFunctionType

    nc = bass.Bass()
    zab_d = nc.dram_tensor("zab", [128, 2 * free], dtb, kind="ExternalInput")
    acc_d = nc.dram_tensor("acc", [128, 1], dtf, kind="ExternalOutput")

    with (
        nc.sbuf_tensor([128, 2 * free], dtb) as zab_sb,
        nc.sbuf_tensor([128, free], dtf) as t_sb,
        nc.sbuf_tensor([128, free], dtf) as s_sb,
        nc.sbuf_tensor([128, 1], dtf) as r_sb,
        nc.sbuf_tensor([1, 1], dtf) as scratch_sb,
        nc.semaphore() as dsem,
        nc.semaphore() as csem,
        nc.Block() as block,
    ):
        @block.sync
        def _(sync):
            sync.dma_start(zab_sb[:, :free],
                           zab_d[:, :free]).then_inc(dsem, 16)
            sync.wait_ge(csem, 6)
            sync.dma_start(acc_d[:], r_sb[:]).then_inc(dsem, 16)

        @block.vector
        def _(vector):
            nc.vector.memset(scratch_sb[:], 0.0).then_inc(csem, 4)
            vector.wait_ge(dsem, 32)
            nc.vector.tensor_add(t_sb[:], zab_sb[:, :free],
                                 zab_sb[:, free:]).then_inc(csem, 1)

        @block.scalar
        def _(scalar):
            # zb half on the scalar HWDGE ring: parallel issuance + queues
            nc.scalar.dma_start(zab_sb[:, free:],
                                zab_d[:, free:]).then_inc(dsem, 16)
            # preload the sigmoid act table while the DMA is in flight
            scalar.wait_ge(csem, 4)
            nc.scalar.activation(scratch_sb[:], scratch_sb[:], AF.Sigmoid)
            scalar.wait_ge(csem, 5)
            nc.scalar.activation(s_sb[:], t_sb[:], AF.Sigmoid, scale=0.5,
                                 accum_out=r_sb[:]).then_inc(csem, 1)

    return nc


def _get_prog(key, builder, *args):
    if key not in _PROGS:
        _PROGS[key] = builder(*args)
    return _PROGS[key]


def _run(nc, in_maps, tag):
    from concourse.bass_utils import run_bass_kernel_spmd
    import time

    t0 = time.perf_counter()
    res = run_bass_kernel_spmd(nc, in_maps, list(range(NCORES)), trace=TRACE)
    LAST[tag + "_wall_s"] = time.perf_counter() - t0
    LAST[tag + "_exec_ns"] = res.exec_time_ns
    return res.results


def kernel(x, w1, w2, inc_rows, inc_cols, n_nodes=None, n_edges=None):
    x = np.asarray(x, dtype=np.float32)
    w1 = np.asarray(w1, dtype=np.float32)
    w2 = np.asarray(w2, dtype=np.float32)
    inc_rows = np.asarray(inc_rows)
    inc_cols = np.asarray(inc_cols)
    assert x.shape == (N_EDGES, C) and inc_rows.shape == (NNZ,)
    assert np.array_equal(inc_cols.astype(np.int64),
                          np.arange(NNZ, dtype=np.int64) // 2)

    # ---- host prep: degree-sorted node placement ----
    rs = inc_rows.astype(np.int64)
    deg = np.bincount(rs, minlength=NODES_PAD)      # padded node space
    order = np.argsort(-deg, kind="stable")         # node rank by deg desc
    rank = np.empty(NODES_PAD, np.int64)
    rank[order] = np.arange(NODES_PAD)
    ds = deg[order]                                 # sorted degrees

    win = rank >> 7                                 # window of each node
    lane = rank & 127
    core_of = win % NCORES
    slot_of = win // NCORES

    rjs = ds.reshape(NWIN, BLK).max(1).reshape(NSLOT, NCORES).max(1)
    rjs = np.maximum(rjs, 1).astype(np.int64)

    groups = _make_groups(rjs)
    blocks, ncol = _pair_blocks(groups)

    # flat column start for (slot, t): where the t-th nnz block of each
    # slot's 128 lanes lives
    maxrj = int(rjs.max())
    slot_t_col = np.full((NSLOT, maxrj), -1, np.int64)
    for (s0, w, rj) in groups:
        # recompute this group's colbase from blocks: first block of group
        pass
    col = 0
    for (s0, w, rj) in groups:
        npair = rj // 2
        odd = rj % 2
        for p in range(npair):
            for i in range(2):
                t = 2 * p + i
                for ws in range(w):
                    slot_t_col[s0 + ws, t] = col + i * w * BLK + ws * BLK
            col += 2 * w * BLK
        if odd:
            t = rj - 1
            for ws in range(w):
                slot_t_col[s0 + ws, t] = col + ws * BLK
            col += w * BLK
    assert col == ncol

    # per-nnz placement: sort nnz by node to get within-node index t
    nnz_order = np.argsort(rs, kind="stable")
    rs_s = rs[nnz_order]
    cs_s = inc_cols.astype(np.int64)[nnz_order]
    starts = np.zeros(NODES_PAD, np.int64)
    starts[1:] = np.cumsum(deg)[:-1]
    t_k = np.arange(NNZ, dtype=np.int64) - starts[rs_s]

    core_k = core_of[rs_s]
    col_k = slot_t_col[slot_of[rs_s], t_k] + lane[rs_s]

    x8 = x.astype(ml_dtypes.float8_e4m3)
    XF = np.zeros((NCORES, ncol, C), dtype=ml_dtypes.float8_e4m3)
    XF[core_k, col_k, :] = x8[cs_s]
    XFT = np.ascontiguousarray(XF.transpose(0, 2, 1))   # [8, 128, ncol]

    w1b = w1.astype(ml_dtypes.bfloat16)
    w2cb = w2[:, 0:1].astype(ml_dtypes.bfloat16)
    eye = (np.arange(128)[:, None] == np.arange(128)[None, :])
    i2h = np.broadcast_to(eye[:, None, :], (128, 2, 128)).astype(
        ml_dtypes.float8_e4m3)
    i2h = np.ascontiguousarray(i2h)

    prog_a = _get_prog(("A", tuple(rjs.tolist())), _build_prog_a, rjs)
    in_maps = [{"xg": XFT[m], "i2": i2h, "w1": w1b, "w2c": w2cb}
               for m in range(NCORES)]
    res_a = _run(prog_a, in_maps, "A")

    # ---- host glue: assemble hw0, gather per-nonzero values ----
    parts = np.stack([res_a[m]["hw0"].reshape(-1) for m in range(NCORES)])
    # node n -> parts[core_of[n], slot_of[n]*128 + lane[n]]
    hw0 = parts[core_of, slot_of * BLK + lane]          # [NODES_PAD]
    zg = hw0[rs]
    za = zg[0::2]
    zb = zg[1::2]

    # ---- launch B: sigmoid + reduce ----
    FREE = -(-N_EDGES // (NCORES * 128))               # 196
    tot = NCORES * 128 * FREE
    zap = np.full(tot, -1.0e4, np.float32)
    zbp = np.full(tot, -1.0e4, np.float32)
    zap[:N_EDGES] = za
    zbp[:N_EDGES] = zb
    zab = np.concatenate(
        [zap.reshape(NCORES, 128, FREE), zbp.reshape(NCORES, 128, FREE)],
        axis=2).astype(ml_dtypes.bfloat16)

    prog_b = _get_prog(("B", FREE), _build_prog_b, FREE)
    in_maps_b = [{"zab": zab[m]} for m in range(NCORES)]
    res_b = _run(prog_b, in_maps_b, "B")

    total = float(sum(float(r["acc"].sum()) for r in res_b))
    return np.array(total / N_EDGES, dtype=np.float32)
